# revision 4
# baseline (speedup 1.0000x reference)
"""Trainium2 Bass kernel for EnhancedTemporalAttentionLayer.

Computation (reference):
  x = nf @ W_in + b_in                     [4096, 256]
  q,k,v = per-head projections of x        [4, 4096, 64]
  scores = q k^T / 8 + time_bias(dt)       [4, 4096, 4096]
  scores = where(edge_blocked, -1e9, scores)
  out = softmax(scores) @ v -> [4096, 256] @ Wo + bo

Strategy: sequence-parallel over 8 cores (512 query rows each). Each core
redundantly computes full K/V (cheap), its own Q rows, then a pipelined
attention loop over 32 key-blocks of 128:
  PE:  S^T[s,r] = K_h^T-block.T @ Q_h^T        (f32r, full PE rate)
  DVE: S^T += bias^T-block                     (combined edge/time bias, host-built)
  ACT: E^T = exp(S^T) -> SBUF (f32r)
  PE:  O^T[65,512] += [V_block | 1].T @ E^T    (row 64 accumulates softmax sums)
then out^T_h = O^T[0:64] * broadcast(1/O^T[64]) and y = sum_h out_h @ Wo_h + bo.
Softmax needs no max-subtraction: |scores| <= ~40 and blocked entries underflow
exp to exactly 0.
"""
import numpy as np
from contextlib import ExitStack

import concourse.bass as bass
from concourse import bacc
import concourse.mybir as mybir
import concourse.tile as tile

L = 4096
IN = 45
HID = 256
NH = 4
DH = 64
NCORES = 8
R = L // NCORES          # 512 query rows per core
SB = 128                 # key-block size
NSB = L // SB            # 32 key blocks
NEG = np.float32(-1e9)
BUCKETS = ((0.0, 5.0, 0.2), (5.0, 15.0, 0.1), (15.0, 60.0, 0.0), (60.0, 240.0, -0.1))

F32 = mybir.dt.float32
F32R = mybir.dt.float32r
EXPF = mybir.ActivationFunctionType.Exp
IDF = mybir.ActivationFunctionType.Identity


def build_program():
    nc = bacc.Bacc()

    nf_t = nc.declare_dram_parameter("nf_t", [IN + 1, L], F32R, isOutput=False)
    nfr_t = nc.declare_dram_parameter("nfr_t", [IN + 1, R], F32R, isOutput=False)
    w_in = nc.declare_dram_parameter("w_in", [IN + 1, HID], F32R, isOutput=False)
    wq = nc.declare_dram_parameter("wq", [2, 128, HID], F32R, isOutput=False)
    wk = nc.declare_dram_parameter("wk", [2, 128, HID], F32R, isOutput=False)
    wv = nc.declare_dram_parameter("wv", [2, 128, HID], F32R, isOutput=False)
    wo = nc.declare_dram_parameter("wo", [NH, DH, HID], F32R, isOutput=False)
    bqd = nc.declare_dram_parameter("bqd", [DH, NH], F32, isOutput=False)
    bkd = nc.declare_dram_parameter("bkd", [DH, NH], F32, isOutput=False)
    bvd = nc.declare_dram_parameter("bvd", [1, HID], F32, isOutput=False)
    bod = nc.declare_dram_parameter("bod", [1, HID], F32, isOutput=False)
    onesd = nc.declare_dram_parameter("onesd", [1, 128], F32R, isOutput=False)
    biasT = nc.declare_dram_parameter("biasT", [SB, NSB, R], F32, isOutput=False)
    y = nc.declare_dram_parameter("y", [R, HID], F32, isOutput=True)

    with tile.TileContext(nc) as tc, ExitStack() as ctx:
        persist = ctx.enter_context(tc.tile_pool(name="persist", bufs=1))

        k_t = persist.tile([DH, NH, L], F32R)            # K^T  [64, h, 4096]
        v_t = persist.tile([SB, NSB, NH, DH + 1], F32R)  # [V | ones]
        q_t = persist.tile([DH, NH, R], F32R)
        o_t = persist.tile([DH, NH, R], F32R)            # normalized out^T
        wo_t = persist.tile([DH, NH, HID], F32R)
        bq_t = persist.tile([DH, NH], F32)
        bk_t = persist.tile([DH, NH], F32)
        bv_t = persist.tile([SB, HID], F32)
        bo_t = persist.tile([SB, HID], F32)
        ones_t = persist.tile([1, 128], F32R)

        nc.sync.dma_start(out=bq_t, in_=bqd[:, :])
        nc.sync.dma_start(out=bk_t, in_=bkd[:, :])
        nc.sync.dma_start(out=ones_t, in_=onesd[:, :])
        bvd_ap = bvd[:, :]
        nc.sync.dma_start(
            out=bv_t,
            in_=bass.AP(tensor=bvd_ap.tensor, offset=bvd_ap.offset,
                        ap=[[0, SB]] + bvd_ap.ap[1:]),
        )
        bod_ap = bod[:, :]
        nc.sync.dma_start(
            out=bo_t,
            in_=bass.AP(tensor=bod_ap.tensor, offset=bod_ap.offset,
                        ap=[[0, SB]] + bod_ap.ap[1:]),
        )
        for h in range(NH):
            nc.sync.dma_start(out=wo_t[:, h, :], in_=wo[h])
        # ones column of V_aug: broadcast 1.0 into [:, :, :, 64]
        ones_ap = onesd[:, :]
        nc.sync.dma_start(
            out=v_t[:, :, :, DH:DH + 1],
            in_=bass.AP(tensor=ones_ap.tensor, offset=ones_ap.offset,
                        ap=[[0, SB], [1, NSB * NH]]),
        )

        with tc.tile_pool(name="proj", bufs=1) as proj, \
             tc.tile_pool(name="pps", bufs=2, space="PSUM") as pps:
            nf_tile = proj.tile([IN + 1, L], F32R)
            nfr_tile = proj.tile([IN + 1, R], F32R)
            w_in_t = proj.tile([IN + 1, HID], F32R)
            wq_t = proj.tile([128, 2, HID], F32R)
            wk_t = proj.tile([128, 2, HID], F32R)
            wv_t = proj.tile([128, 2, HID], F32R)
            x_t = proj.tile([128, 2, L], F32R)
            xo_t = proj.tile([128, 2, R], F32R)

            nc.sync.dma_start(out=nf_tile, in_=nf_t[:, :])
            nc.sync.dma_start(out=nfr_tile, in_=nfr_t[:, :])
            nc.sync.dma_start(out=w_in_t, in_=w_in[:, :])
            for cb in range(2):
                nc.sync.dma_start(out=wq_t[:, cb, :], in_=wq[cb])
                nc.sync.dma_start(out=wk_t[:, cb, :], in_=wk[cb])
                nc.sync.dma_start(out=wv_t[:, cb, :], in_=wv[cb])

            # x^T = W_in_aug.T @ nf_aug^T   [256, 4096] (+ own-rows copy)
            for cb in range(2):
                for sc in range(L // 512):
                    xp = pps.tile([128, 512], F32, tag="xp")
                    nc.tensor.matmul(
                        xp, w_in_t[:, cb * 128:(cb + 1) * 128],
                        nf_tile[:, sc * 512:(sc + 1) * 512],
                        start=True, stop=True)
                    nc.scalar.copy(out=x_t[:, cb, sc * 512:(sc + 1) * 512], in_=xp)
                xp = pps.tile([128, 512], F32, tag="xp")
                nc.tensor.matmul(xp, w_in_t[:, cb * 128:(cb + 1) * 128],
                                 nfr_tile, start=True, stop=True)
                nc.scalar.copy(out=xo_t[:, cb, :], in_=xp)

            # K^T per head: [64, 4096]; Q^T per head (own rows): [64, 512]
            for h in range(NH):
                hs = slice(h * DH, (h + 1) * DH)
                for sc in range(L // 512):
                    kp = pps.tile([DH, 512], F32, tag="kp")
                    for cb in range(2):
                        nc.tensor.matmul(
                            kp, wk_t[:, cb, hs],
                            x_t[:, cb, sc * 512:(sc + 1) * 512],
                            start=(cb == 0), stop=(cb == 1))
                    nc.scalar.activation(
                        out=k_t[:, h, sc * 512:(sc + 1) * 512], in_=kp,
                        func=IDF, bias=bk_t[:, h:h + 1], scale=1.0)
                qp = pps.tile([DH, 512], F32, tag="kp")
                for cb in range(2):
                    nc.tensor.matmul(qp, wq_t[:, cb, hs], xo_t[:, cb, :],
                                     start=(cb == 0), stop=(cb == 1))
                nc.scalar.activation(out=q_t[:, h, :], in_=qp,
                                     func=IDF, bias=bq_t[:, h:h + 1], scale=1.0)

            # V rows (+bv) into [V | ones] layout
            for sb in range(NSB):
                vp = pps.tile([SB, HID], F32, tag="vp")
                for cb in range(2):
                    nc.tensor.matmul(vp, x_t[:, cb, sb * SB:(sb + 1) * SB],
                                     wv_t[:, cb, :], start=(cb == 0), stop=(cb == 1))
                for h in range(NH):
                    hs = slice(h * DH, (h + 1) * DH)
                    nc.vector.tensor_add(out=v_t[:, sb, h, 0:DH],
                                         in0=vp[:, hs], in1=bv_t[:, hs])

        with tc.tile_pool(name="attn", bufs=1) as attnp, \
             tc.tile_pool(name="sp_ps", bufs=3, space="PSUM") as spool, \
             tc.tile_pool(name="op_ps", bufs=2, space="PSUM") as opool, \
             tc.tile_pool(name="iv_ps", bufs=1, space="PSUM") as ivpool, \
             tc.tile_pool(name="y_ps", bufs=1, space="PSUM") as ypsp, \
             tc.tile_pool(name="ering", bufs=3) as ering, \
             tc.tile_pool(name="small", bufs=2) as small:
            bias_t = attnp.tile([SB, NSB, R], F32)
            for g in range(8):
                nc.sync.dma_start(out=bias_t[:, g * 4:(g + 1) * 4, :],
                                  in_=biasT[:, g * 4:(g + 1) * 4, :])

            for h in range(NH):
                op = opool.tile([DH + 1, R], F32, tag="op")
                for sb in range(NSB):
                    sp = spool.tile([SB, R], F32, tag="sp")
                    nc.tensor.matmul(sp, k_t[:, h, sb * SB:(sb + 1) * SB],
                                     q_t[:, h, :], start=True, stop=True)
                    nc.vector.tensor_add(out=sp, in0=sp, in1=bias_t[:, sb, :])
                    et = ering.tile([SB, R], F32R, tag="et")
                    nc.scalar.activation(out=et, in_=sp, func=EXPF)
                    nc.tensor.matmul(op, v_t[:, sb, h, :], et,
                                     start=(sb == 0), stop=(sb == NSB - 1))
                inv = small.tile([1, R], F32R, tag="inv")
                with nc.allow_low_precision(reason="f32r rounding of softmax reciprocal"):
                    nc.vector.reciprocal(out=inv, in_=op[DH:DH + 1, :])
                ib = ivpool.tile([DH, R], F32, tag="ib")
                nc.tensor.matmul(ib, ones_t[0:1, 0:DH], inv, start=True, stop=True)
                ibs = small.tile([DH, R], F32R, tag="ibs")
                nc.scalar.copy(out=ibs, in_=ib)
                nc.vector.tensor_mul(out=o_t[:, h, :], in0=op[0:DH, :], in1=ibs)

            for rt in range(R // SB):
                yp = ypsp.tile([SB, HID], F32, tag="yp")
                for h in range(NH):
                    nc.tensor.matmul(yp, o_t[:, h, rt * SB:(rt + 1) * SB],
                                     wo_t[:, h, :], start=(h == 0), stop=(h == NH - 1))
                ys = small.tile([SB, HID], F32, tag="ys")
                nc.vector.tensor_add(out=ys, in0=yp, in1=bo_t)
                nc.scalar.dma_start(out=y[rt * SB:(rt + 1) * SB, :], in_=ys)

    nc.compile()
    return nc


def prepare_inputs(node_features, temporal_data, edge_index,
                   W_in, b_in, Wq, bq, Wk, bk, Wv, bv, Wo, bo):
    """Host-side prep: layouts, bias matrix, per-core sharding."""
    f32 = np.float32
    nf = np.asarray(node_features, f32)
    W_in = np.asarray(W_in, f32)
    b_in = np.asarray(b_in, f32)
    Wq = np.asarray(Wq, f32)
    bq = np.asarray(bq, f32)
    Wk = np.asarray(Wk, f32)
    bk = np.asarray(bk, f32)
    Wv = np.asarray(Wv, f32)
    bv = np.asarray(bv, f32)
    Wo = np.asarray(Wo, f32)
    bo = np.asarray(bo, f32)
    edge_index = np.asarray(edge_index)

    scale = f32(1.0 / np.sqrt(DH))

    nf_aug_t = np.concatenate([nf.T, np.ones((1, L), f32)], axis=0)   # [46, L]
    w_in_aug = np.concatenate([W_in, b_in[None, :]], axis=0)          # [46, 256]
    wq_s = (Wq * scale).reshape(2, 128, HID)
    wk_r = Wk.reshape(2, 128, HID)
    wv_r = Wv.reshape(2, 128, HID)
    wo_r = Wo.reshape(NH, DH, HID)
    bq_s = (bq * scale).reshape(NH, DH).T.copy()                      # [64, 4]
    bk_r = bk.reshape(NH, DH).T.copy()
    ones128 = np.ones((1, 128), f32)

    # combined additive bias (time bias where allowed, -1e9 where blocked)
    dt = np.ascontiguousarray(np.asarray(temporal_data[:, :, 0], f32))
    tb = np.zeros_like(dt)
    for lo, hi, w in BUCKETS:
        if w != 0.0:
            tb += f32(w) * ((dt >= f32(lo)) & (dt <= f32(hi))).astype(f32)
    allowed = np.zeros((L, L), dtype=bool)
    allowed[edge_index[0], edge_index[1]] = True
    np.fill_diagonal(allowed, True)
    biasf = np.where(allowed, tb, NEG)            # [r, s]
    biasf_T = np.ascontiguousarray(biasf.T)       # [s, r]

    in_maps = []
    for c in range(NCORES):
        rows = slice(c * R, (c + 1) * R)
        bt = biasf_T[:, rows]                                  # [4096, 512]
        bt = np.ascontiguousarray(
            bt.reshape(NSB, SB, R).transpose(1, 0, 2))         # [128, 32, 512]
        in_maps.append({
            "nf_t": nf_aug_t,
            "nfr_t": np.ascontiguousarray(nf_aug_t[:, rows]),
            "w_in": w_in_aug,
            "wq": wq_s, "wk": wk_r, "wv": wv_r, "wo": wo_r,
            "bqd": bq_s, "bkd": bk_r,
            "bvd": bv[None, :], "bod": bo[None, :],
            "onesd": ones128,
            "biasT": bt,
        })
    return in_maps


_PROGRAM = None


def kernel(**inputs) -> np.ndarray:
    global _PROGRAM
    from concourse.bass_utils import run_bass_kernel_spmd

    if _PROGRAM is None:
        _PROGRAM = build_program()
    in_maps = prepare_inputs(**inputs)
    res = run_bass_kernel_spmd(_PROGRAM, in_maps, list(range(NCORES))).results
    return np.concatenate([r["y"] for r in res], axis=0)


# revision 9
# speedup vs baseline: 1.4264x; 1.4264x over previous
"""Trainium2 Bass kernel for EnhancedTemporalAttentionLayer.

Computation (reference):
  x = nf @ W_in + b_in                     [4096, 256]
  q,k,v = per-head projections of x        [4, 4096, 64]
  scores = q k^T / 8 + time_bias(dt)       [4, 4096, 4096]
  scores = where(edge_blocked, -1e9, scores)
  out = softmax(scores) @ v -> [4096, 256] @ Wo + bo

Strategy: sequence-parallel over 8 cores (512 query rows each). Each core
redundantly computes full K/V (cheap), its own Q rows, then a pipelined
attention loop over 32 key-blocks of 128, processing head PAIRS so the
elementwise engines see wide [128, 1024] ops:
  PE:  S^T[s, r(2 heads)] = K_h-block.T @ Q_h^T   x2   (f32r, full PE rate)
  DVE: S^T += bias^T-block (repeated across the pair)
  ACT: E^T = exp(S^T) -> SBUF f32r
  PE:  O^T_h[65,512] += [V_block | 1].T @ E^T_h   x2   (row 64 = softmax sums)
then out^T_h = O^T[0:64] * broadcast(1/O^T[64]) and y = sum_h out_h @ Wo_h + bo.
The 1/sqrt(64) scale is folded into Wq; the combined edge/time bias is built on
host. Softmax needs no max-subtraction: |scores| <= ~40 and blocked entries
underflow exp to exactly 0.
"""
import numpy as np
from contextlib import ExitStack

import concourse.bass as bass
from concourse import bacc
import concourse.mybir as mybir
import concourse.tile as tile

L = 4096
IN = 45
HID = 256
NH = 4
DH = 64
NCORES = 8
R = L // NCORES          # 512 query rows per core
SB = 128                 # key-block size
NSB = L // SB            # 32 key blocks
NEG = np.float32(-1e9)
BUCKETS = ((0.0, 5.0, 0.2), (5.0, 15.0, 0.1), (15.0, 60.0, 0.0), (60.0, 240.0, -0.1))

F32 = mybir.dt.float32
F32R = mybir.dt.float32r
EXPF = mybir.ActivationFunctionType.Exp
ADD = None  # set lazily


def build_program(reps=1, dma_in_loop=True):
    """reps>1 wraps the attention+output phase (incl. bias DMA) in a hardware
    loop — used only for timing measurements (amortizes host dispatch jitter)."""
    from concourse.alu_op_type import AluOpType
    nc = bacc.Bacc()

    nf_t = nc.declare_dram_parameter("nf_t", [IN + 1, L], F32R, isOutput=False)
    nfr_t = nc.declare_dram_parameter("nfr_t", [IN + 1, R], F32R, isOutput=False)
    w_in = nc.declare_dram_parameter("w_in", [IN + 1, HID], F32R, isOutput=False)
    wq = nc.declare_dram_parameter("wq", [2, 128, HID], F32R, isOutput=False)
    wk = nc.declare_dram_parameter("wk", [2, 128, HID], F32R, isOutput=False)
    wv = nc.declare_dram_parameter("wv", [2, 128, HID], F32R, isOutput=False)
    wo = nc.declare_dram_parameter("wo", [NH, DH, HID], F32R, isOutput=False)
    bqd = nc.declare_dram_parameter("bqd", [128, 2], F32, isOutput=False)
    bkd = nc.declare_dram_parameter("bkd", [128, 2], F32, isOutput=False)
    bvd = nc.declare_dram_parameter("bvd", [1, HID], F32, isOutput=False)
    bod = nc.declare_dram_parameter("bod", [1, HID], F32, isOutput=False)
    onesd = nc.declare_dram_parameter("onesd", [1, 128], F32R, isOutput=False)
    biasT = nc.declare_dram_parameter("biasT", [SB, NSB, R], F32, isOutput=False)
    y = nc.declare_dram_parameter("y", [R, HID], F32, isOutput=True)

    with tile.TileContext(nc) as tc, ExitStack() as ctx:
        persist = ctx.enter_context(tc.tile_pool(name="persist", bufs=1))

        # head-pair packed: partitions [0:64]=head 2g, [64:128]=head 2g+1
        k_t = persist.tile([128, 2, L], F32R)
        q_t = persist.tile([128, 2, R], F32R)
        v_t = persist.tile([SB, NSB, NH, DH + 1], F32R)  # [V | ones]
        o_t = persist.tile([DH, NH, R], F32R)            # normalized out^T
        wo_t = persist.tile([DH, NH, HID], F32R)
        bq_t = persist.tile([128, 2], F32)
        bk_t = persist.tile([128, 2], F32)
        bv_t = persist.tile([SB, HID], F32)
        bo_t = persist.tile([SB, HID], F32)
        ones_t = persist.tile([1, 128], F32R)

        nc.sync.dma_start(out=bq_t, in_=bqd[:, :])
        nc.sync.dma_start(out=bk_t, in_=bkd[:, :])
        nc.sync.dma_start(out=ones_t, in_=onesd[:, :])
        bvd_ap = bvd[:, :]
        nc.sync.dma_start(
            out=bv_t,
            in_=bass.AP(tensor=bvd_ap.tensor, offset=bvd_ap.offset,
                        ap=[[0, SB]] + bvd_ap.ap[1:]),
        )
        bod_ap = bod[:, :]
        nc.sync.dma_start(
            out=bo_t,
            in_=bass.AP(tensor=bod_ap.tensor, offset=bod_ap.offset,
                        ap=[[0, SB]] + bod_ap.ap[1:]),
        )
        for h in range(NH):
            nc.sync.dma_start(out=wo_t[:, h, :], in_=wo[h])
        # ones column of V_aug: broadcast 1.0 into [:, :, :, 64]
        ones_ap = onesd[:, :]
        nc.sync.dma_start(
            out=v_t[:, :, :, DH:DH + 1],
            in_=bass.AP(tensor=ones_ap.tensor, offset=ones_ap.offset,
                        ap=[[0, SB], [1, NSB * NH]]),
        )

        with tc.tile_pool(name="proj", bufs=1) as proj, \
             tc.tile_pool(name="pps", bufs=2, space="PSUM") as pps:
            nf_tile = proj.tile([IN + 1, L], F32R)
            nfr_tile = proj.tile([IN + 1, R], F32R)
            w_in_t = proj.tile([IN + 1, HID], F32R)
            wq_t = proj.tile([128, 2, HID], F32R)
            wk_t = proj.tile([128, 2, HID], F32R)
            wv_t = proj.tile([128, 2, HID], F32R)
            x_t = proj.tile([128, 2, L], F32R)
            xo_t = proj.tile([128, 2, R], F32R)

            nc.sync.dma_start(out=nf_tile, in_=nf_t[:, :])
            nc.sync.dma_start(out=nfr_tile, in_=nfr_t[:, :])
            nc.sync.dma_start(out=w_in_t, in_=w_in[:, :])
            for cb in range(2):
                nc.sync.dma_start(out=wq_t[:, cb, :], in_=wq[cb])
                nc.sync.dma_start(out=wk_t[:, cb, :], in_=wk[cb])
                nc.sync.dma_start(out=wv_t[:, cb, :], in_=wv[cb])

            # x^T = W_in_aug.T @ nf_aug^T   [256, 4096] (+ own-rows copy)
            for cb in range(2):
                for sc in range(L // 512):
                    xp = pps.tile([128, 512], F32, tag="xp")
                    nc.tensor.matmul(
                        xp, w_in_t[:, cb * 128:(cb + 1) * 128],
                        nf_tile[:, sc * 512:(sc + 1) * 512],
                        start=True, stop=True)
                    nc.scalar.copy(out=x_t[:, cb, sc * 512:(sc + 1) * 512], in_=xp)
                xp = pps.tile([128, 512], F32, tag="xp")
                nc.tensor.matmul(xp, w_in_t[:, cb * 128:(cb + 1) * 128],
                                 nfr_tile, start=True, stop=True)
                nc.scalar.copy(out=xo_t[:, cb, :], in_=xp)

            # K^T / Q^T head-pair packed: pair g occupies partitions
            # [0:64]=head 2g, [64:128]=head 2g+1  (wk columns g*128:(g+1)*128)
            for g in range(2):
                gs = slice(g * 128, (g + 1) * 128)
                for sc in range(L // 512):
                    kp = pps.tile([128, 512], F32, tag="kp")
                    for cb in range(2):
                        nc.tensor.matmul(
                            kp, wk_t[:, cb, gs],
                            x_t[:, cb, sc * 512:(sc + 1) * 512],
                            start=(cb == 0), stop=(cb == 1))
                    nc.vector.tensor_scalar_add(
                        k_t[:, g, sc * 512:(sc + 1) * 512], kp, bk_t[:, g:g + 1])
                qp = pps.tile([128, 512], F32, tag="kp")
                for cb in range(2):
                    nc.tensor.matmul(qp, wq_t[:, cb, gs], xo_t[:, cb, :],
                                     start=(cb == 0), stop=(cb == 1))
                nc.vector.tensor_scalar_add(q_t[:, g, :], qp, bq_t[:, g:g + 1])

            # V rows (+bv) into [V | ones] layout
            for sb in range(NSB):
                vp = pps.tile([SB, HID], F32, tag="vp")
                for cb in range(2):
                    nc.tensor.matmul(vp, x_t[:, cb, sb * SB:(sb + 1) * SB],
                                     wv_t[:, cb, :], start=(cb == 0), stop=(cb == 1))
                nc.vector.tensor_add(out=v_t[:, sb, :, 0:DH], in0=vp, in1=bv_t)

        with tc.tile_pool(name="attn", bufs=1) as attnp, \
             tc.tile_pool(name="sp_ps", bufs=3, space="PSUM") as spool, \
             tc.tile_pool(name="op_ps", bufs=1, space="PSUM") as opool, \
             tc.tile_pool(name="ering", bufs=3) as ering, \
             tc.tile_pool(name="small", bufs=2) as small:
            bias_t = attnp.tile([SB, NSB, R], F32)

            def bias_dma():
                for gd in range(8):
                    nc.sync.dma_start(out=bias_t[:, gd * 4:(gd + 1) * 4, :],
                                      in_=biasT[:, gd * 4:(gd + 1) * 4, :])

            def attn_body(_iv=None):
                if dma_in_loop:
                    bias_dma()
                for g in range(2):
                    h0, h1 = 2 * g, 2 * g + 1
                    op0 = opool.tile([DH + 1, R], F32, tag="op0")
                    op1 = opool.tile([DH + 1, R], F32, tag="op1")
                    for sb in range(NSB):
                        ss = slice(sb * SB, (sb + 1) * SB)
                        sp = spool.tile([SB, 2 * R], F32, tag="sp")
                        nc.tensor.matmul(sp[:, 0:R], k_t[0:DH, g, ss],
                                         q_t[0:DH, g, :], start=True, stop=True)
                        nc.tensor.matmul(sp[:, R:2 * R], k_t[DH:SB, g, ss],
                                         q_t[DH:SB, g, :], start=True, stop=True)
                        bap = bias_t[:, sb, :]
                        brep = bass.AP(tensor=bap.tensor, offset=bap.offset,
                                       ap=[bap.ap[0], [0, 2]] + bap.ap[1:])
                        nc.vector.tensor_add(out=sp, in0=sp, in1=brep)
                        et = ering.tile([SB, 2 * R], F32R, tag="et")
                        nc.scalar.activation(out=et, in_=sp, func=EXPF)
                        nc.tensor.matmul(op0, v_t[:, sb, h0, :], et[:, 0:R],
                                         start=(sb == 0), stop=(sb == NSB - 1))
                        nc.tensor.matmul(op1, v_t[:, sb, h1, :], et[:, R:2 * R],
                                         start=(sb == 0), stop=(sb == NSB - 1))
                    for j, op in ((0, op0), (1, op1)):
                        h = 2 * g + j
                        inv = small.tile([1, R], F32R, tag="inv")
                        with nc.allow_low_precision(reason="f32r rounding of softmax recip"):
                            nc.vector.reciprocal(out=inv, in_=op[DH:DH + 1, :])
                        ib = spool.tile([DH, R], F32, tag="sp")
                        nc.tensor.matmul(ib, ones_t[0:1, 0:DH], inv, start=True, stop=True)
                        ibs = small.tile([DH, R], F32R, tag="ibs")
                        nc.scalar.copy(out=ibs, in_=ib)
                        nc.vector.tensor_mul(out=o_t[:, h, :], in0=op[0:DH, :], in1=ibs)

                for rt in range(R // SB):
                    yp = spool.tile([SB, HID], F32, tag="sp")
                    for h in range(NH):
                        nc.tensor.matmul(yp, o_t[:, h, rt * SB:(rt + 1) * SB],
                                         wo_t[:, h, :], start=(h == 0), stop=(h == NH - 1))
                    ys = small.tile([SB, HID], F32, tag="ys")
                    nc.vector.tensor_add(out=ys, in0=yp, in1=bo_t)
                    nc.scalar.dma_start(out=y[rt * SB:(rt + 1) * SB, :], in_=ys)

            if reps == 1:
                attn_body()
            else:
                if not dma_in_loop:
                    bias_dma()
                with tc.For_i(0, reps, 1) as iv:
                    attn_body(iv)

    nc.compile()
    return nc


def prepare_inputs(node_features, temporal_data, edge_index,
                   W_in, b_in, Wq, bq, Wk, bk, Wv, bv, Wo, bo):
    """Host-side prep: layouts, bias matrix, per-core sharding."""
    f32 = np.float32
    nf = np.asarray(node_features, f32)
    W_in = np.asarray(W_in, f32)
    b_in = np.asarray(b_in, f32)
    Wq = np.asarray(Wq, f32)
    bq = np.asarray(bq, f32)
    Wk = np.asarray(Wk, f32)
    bk = np.asarray(bk, f32)
    Wv = np.asarray(Wv, f32)
    bv = np.asarray(bv, f32)
    Wo = np.asarray(Wo, f32)
    bo = np.asarray(bo, f32)
    edge_index = np.asarray(edge_index)

    scale = f32(1.0 / np.sqrt(DH))

    nf_aug_t = np.concatenate([nf.T, np.ones((1, L), f32)], axis=0)   # [46, L]
    w_in_aug = np.concatenate([W_in, b_in[None, :]], axis=0)          # [46, 256]
    wq_s = (Wq * scale).reshape(2, 128, HID)
    wk_r = Wk.reshape(2, 128, HID)
    wv_r = Wv.reshape(2, 128, HID)
    wo_r = Wo.reshape(NH, DH, HID)
    # head-pair packed per-partition biases: [128, 2] (pair g in column g)
    bq_s = np.ascontiguousarray((bq * scale).reshape(2, 128).T)
    bk_r = np.ascontiguousarray(bk.reshape(2, 128).T)
    ones128 = np.ones((1, 128), f32)

    # combined additive bias (time bias where allowed, -1e9 where blocked)
    dt = np.ascontiguousarray(np.asarray(temporal_data[:, :, 0], f32))
    tb = np.zeros_like(dt)
    for lo, hi, w in BUCKETS:
        if w != 0.0:
            tb += f32(w) * ((dt >= f32(lo)) & (dt <= f32(hi))).astype(f32)
    allowed = np.zeros((L, L), dtype=bool)
    allowed[edge_index[0], edge_index[1]] = True
    np.fill_diagonal(allowed, True)
    biasf = np.where(allowed, tb, NEG)            # [r, s]
    biasf_T = np.ascontiguousarray(biasf.T)       # [s, r]

    in_maps = []
    for c in range(NCORES):
        rows = slice(c * R, (c + 1) * R)
        bt = biasf_T[:, rows]                                  # [4096, 512]
        bt = np.ascontiguousarray(
            bt.reshape(NSB, SB, R).transpose(1, 0, 2))         # [128, 32, 512]
        in_maps.append({
            "nf_t": nf_aug_t,
            "nfr_t": np.ascontiguousarray(nf_aug_t[:, rows]),
            "w_in": w_in_aug,
            "wq": wq_s, "wk": wk_r, "wv": wv_r, "wo": wo_r,
            "bqd": bq_s, "bkd": bk_r,
            "bvd": bv[None, :], "bod": bo[None, :],
            "onesd": ones128,
            "biasT": bt,
        })
    return in_maps


_PROGRAM = None


def kernel(**inputs) -> np.ndarray:
    global _PROGRAM
    from concourse.bass_utils import run_bass_kernel_spmd

    if _PROGRAM is None:
        _PROGRAM = build_program()
    in_maps = prepare_inputs(**inputs)
    res = run_bass_kernel_spmd(_PROGRAM, in_maps, list(range(NCORES))).results
    return np.concatenate([r["y"] for r in res], axis=0)


# revision 18
# speedup vs baseline: 1437.9744x; 1008.1092x over previous
"""Trainium2 Bass kernel for EnhancedTemporalAttentionLayer.

Computation (reference):
  x = nf @ W_in + b_in                     [4096, 256]
  q,k,v = per-head projections of x        [4, 4096, 64]
  scores = q k^T / 8 + time_bias(dt)       [4, 4096, 4096]
  scores = where(edge_blocked, -1e9, scores)
  out = softmax(scores) @ v -> [4096, 256] @ Wo + bo

Strategy: sequence-parallel over 8 cores (512 query rows each). Each core
redundantly computes full K/V (cheap), its own Q rows, then a pipelined
attention loop over 32 key-blocks of 128, processing head PAIRS so the
elementwise engines see wide [128, 1024] ops:
  PE:  S^T[s, r(2 heads)] = K_h-block.T @ Q_h^T   x2   (f32r, full PE rate)
  DVE: S^T += bias^T-block (repeated across the pair)
  ACT: E^T = exp(S^T) -> SBUF f32r
  PE:  O^T_h[65,512] += [V_block | 1].T @ E^T_h   x2   (row 64 = softmax sums)
then out^T_h = O^T[0:64] * broadcast(1/O^T[64]) and y = sum_h out_h @ Wo_h + bo.
The 1/sqrt(64) scale is folded into Wq; the combined edge/time bias is built on
host. Softmax needs no max-subtraction: |scores| <= ~40 and blocked entries
underflow exp to exactly 0.
"""
import numpy as np
from contextlib import ExitStack

import concourse.bass as bass
from concourse import bacc
import concourse.mybir as mybir
import concourse.tile as tile

L = 4096
IN = 45
HID = 256
NH = 4
DH = 64
NCORES = 8
R = L // NCORES          # 512 query rows per core
SB = 128                 # key-block size
NSB = L // SB            # 32 key blocks
NEG = np.float32(-1e9)
BUCKETS = ((0.0, 5.0, 0.2), (5.0, 15.0, 0.1), (15.0, 60.0, 0.0), (60.0, 240.0, -0.1))

F32 = mybir.dt.float32
F32R = mybir.dt.float32r
EXPF = mybir.ActivationFunctionType.Exp
ADD = None  # set lazily


def build_program(reps=1, dma_in_loop=True):
    """reps>1 wraps the attention+output phase (incl. bias DMA) in a hardware
    loop — used only for timing measurements (amortizes host dispatch jitter)."""
    from concourse.alu_op_type import AluOpType
    nc = bacc.Bacc()

    nf_t = nc.declare_dram_parameter("nf_t", [IN + 1, L], F32R, isOutput=False)
    nfr_t = nc.declare_dram_parameter("nfr_t", [IN + 1, R], F32R, isOutput=False)
    w_in = nc.declare_dram_parameter("w_in", [IN + 1, HID], F32R, isOutput=False)
    wq = nc.declare_dram_parameter("wq", [2, 128, HID], F32R, isOutput=False)
    wk = nc.declare_dram_parameter("wk", [2, 128, HID], F32R, isOutput=False)
    wv = nc.declare_dram_parameter("wv", [2, 128, HID], F32R, isOutput=False)
    wo = nc.declare_dram_parameter("wo", [NH, DH, HID], F32R, isOutput=False)
    bqd = nc.declare_dram_parameter("bqd", [128, 2], F32, isOutput=False)
    bkd = nc.declare_dram_parameter("bkd", [128, 2], F32, isOutput=False)
    bvd = nc.declare_dram_parameter("bvd", [1, HID], F32, isOutput=False)
    bod = nc.declare_dram_parameter("bod", [1, HID], F32, isOutput=False)
    onesd = nc.declare_dram_parameter("onesd", [1, 128], F32R, isOutput=False)
    biasT = nc.declare_dram_parameter("biasT", [SB, NSB, R], F32R, isOutput=False)
    identd = nc.declare_dram_parameter("identd", [128, 128], F32R, isOutput=False)
    y = nc.declare_dram_parameter("y", [R, HID], F32, isOutput=True)

    with tile.TileContext(nc) as tc, ExitStack() as ctx:
        persist = ctx.enter_context(tc.tile_pool(name="persist", bufs=1))

        # head-pair packed: partitions [0:64]=head 2g, [64:128]=head 2g+1
        k_t = persist.tile([128, 2, L], F32R)
        q_t = persist.tile([128, 2, R], F32R)
        v_t = persist.tile([SB, NSB, NH, DH + 1], F32R)  # [V | ones]
        o_t = persist.tile([DH, NH, R], F32R)            # normalized out^T
        wo_t = persist.tile([DH, NH, HID], F32R)
        bq_t = persist.tile([128, 2], F32)
        bk_t = persist.tile([128, 2], F32)
        bv_t = persist.tile([SB, HID], F32)
        bo_t = persist.tile([SB, HID], F32)
        ones_t = persist.tile([1, 128], F32R)
        id_t = persist.tile([128, 128], F32R)

        nc.sync.dma_start(out=bq_t, in_=bqd[:, :])
        nc.sync.dma_start(out=bk_t, in_=bkd[:, :])
        nc.sync.dma_start(out=ones_t, in_=onesd[:, :])
        nc.sync.dma_start(out=id_t, in_=identd[:, :])
        bvd_ap = bvd[:, :]
        nc.sync.dma_start(
            out=bv_t,
            in_=bass.AP(tensor=bvd_ap.tensor, offset=bvd_ap.offset,
                        ap=[[0, SB]] + bvd_ap.ap[1:]),
        )
        bod_ap = bod[:, :]
        nc.sync.dma_start(
            out=bo_t,
            in_=bass.AP(tensor=bod_ap.tensor, offset=bod_ap.offset,
                        ap=[[0, SB]] + bod_ap.ap[1:]),
        )
        for h in range(NH):
            nc.sync.dma_start(out=wo_t[:, h, :], in_=wo[h])
        # ones column of V_aug: broadcast 1.0 into [:, :, :, 64]
        ones_ap = onesd[:, :]
        nc.sync.dma_start(
            out=v_t[:, :, :, DH:DH + 1],
            in_=bass.AP(tensor=ones_ap.tensor, offset=ones_ap.offset,
                        ap=[[0, SB], [1, NSB * NH]]),
        )

        with tc.tile_pool(name="proj", bufs=1) as proj, \
             tc.tile_pool(name="pps", bufs=2, space="PSUM") as pps:
            nf_tile = proj.tile([IN + 1, L], F32R)
            nfr_tile = proj.tile([IN + 1, R], F32R)
            w_in_t = proj.tile([IN + 1, HID], F32R)
            wq_t = proj.tile([128, 2, HID], F32R)
            wk_t = proj.tile([128, 2, HID], F32R)
            wv_t = proj.tile([128, 2, HID], F32R)
            x_t = proj.tile([128, 2, L], F32R)
            xo_t = proj.tile([128, 2, R], F32R)

            nc.sync.dma_start(out=nf_tile, in_=nf_t[:, :])
            nc.sync.dma_start(out=nfr_tile, in_=nfr_t[:, :])
            nc.sync.dma_start(out=w_in_t, in_=w_in[:, :])
            for cb in range(2):
                nc.sync.dma_start(out=wq_t[:, cb, :], in_=wq[cb])
                nc.sync.dma_start(out=wk_t[:, cb, :], in_=wk[cb])
                nc.sync.dma_start(out=wv_t[:, cb, :], in_=wv[cb])

            # x^T = W_in_aug.T @ nf_aug^T   [256, 4096] (+ own-rows copy)
            for cb in range(2):
                for sc in range(L // 512):
                    xp = pps.tile([128, 512], F32, tag="xp")
                    nc.tensor.matmul(
                        xp, w_in_t[:, cb * 128:(cb + 1) * 128],
                        nf_tile[:, sc * 512:(sc + 1) * 512],
                        start=True, stop=True)
                    if sc % 2 == 0:
                        nc.scalar.copy(out=x_t[:, cb, sc * 512:(sc + 1) * 512], in_=xp)
                    else:
                        nc.vector.tensor_copy(out=x_t[:, cb, sc * 512:(sc + 1) * 512], in_=xp)
                xp = pps.tile([128, 512], F32, tag="xp")
                nc.tensor.matmul(xp, w_in_t[:, cb * 128:(cb + 1) * 128],
                                 nfr_tile, start=True, stop=True)
                nc.scalar.copy(out=xo_t[:, cb, :], in_=xp)

            # K^T / Q^T head-pair packed: pair g occupies partitions
            # [0:64]=head 2g, [64:128]=head 2g+1  (wk columns g*128:(g+1)*128)
            for g in range(2):
                gs = slice(g * 128, (g + 1) * 128)
                for sc in range(L // 512):
                    kp = pps.tile([128, 512], F32, tag="kp")
                    for cb in range(2):
                        nc.tensor.matmul(
                            kp, wk_t[:, cb, gs],
                            x_t[:, cb, sc * 512:(sc + 1) * 512],
                            start=(cb == 0), stop=(cb == 1))
                    nc.vector.tensor_scalar_add(
                        k_t[:, g, sc * 512:(sc + 1) * 512], kp, bk_t[:, g:g + 1])
                qp = pps.tile([128, 512], F32, tag="kp")
                for cb in range(2):
                    nc.tensor.matmul(qp, wq_t[:, cb, gs], xo_t[:, cb, :],
                                     start=(cb == 0), stop=(cb == 1))
                nc.vector.tensor_scalar_add(q_t[:, g, :], qp, bq_t[:, g:g + 1])

            # V rows (+bv) into [V | ones] layout
            for sb in range(NSB):
                vp = pps.tile([SB, HID], F32, tag="vp")
                for cb in range(2):
                    nc.tensor.matmul(vp, x_t[:, cb, sb * SB:(sb + 1) * SB],
                                     wv_t[:, cb, :], start=(cb == 0), stop=(cb == 1))
                nc.vector.tensor_add(out=v_t[:, sb, :, 0:DH], in0=vp, in1=bv_t)

        with tc.tile_pool(name="attn", bufs=1) as attnp, \
             tc.tile_pool(name="sp_ps", bufs=3, space="PSUM") as spool, \
             tc.tile_pool(name="op_ps", bufs=1, space="PSUM") as opool, \
             tc.tile_pool(name="ering", bufs=4) as ering, \
             tc.tile_pool(name="small", bufs=2) as small:
            bias_t = attnp.tile([SB, NSB, R], F32R)

            def bias_dma():
                for gd in range(8):
                    nc.sync.dma_start(out=bias_t[:, gd * 4:(gd + 1) * 4, :],
                                      in_=biasT[:, gd * 4:(gd + 1) * 4, :])

            def attn_body(_iv=None):
                if dma_in_loop:
                    bias_dma()
                def emit_qk(g, sb):
                    ss = slice(sb * SB, (sb + 1) * SB)
                    sp = spool.tile([SB, 2 * R], F32, tag="sp")
                    bap = bias_t[:, sb, :]
                    on_pe = (sb % 3 == 1)
                    if on_pe:
                        nc.tensor.matmul(sp[:, 0:R], id_t, bap, start=True,
                                         stop=True, skip_group_check=True)
                        nc.tensor.matmul(sp[:, R:2 * R], id_t, bap, start=True,
                                         stop=True, skip_group_check=True)
                    nc.tensor.matmul(sp[:, 0:R], k_t[0:DH, g, ss],
                                     q_t[0:DH, g, :], start=not on_pe, stop=True,
                                     skip_group_check=True)
                    nc.tensor.matmul(sp[:, R:2 * R], k_t[DH:SB, g, ss],
                                     q_t[DH:SB, g, :], start=not on_pe, stop=True,
                                     skip_group_check=True)
                    return sp, on_pe

                for g in range(2):
                    h0, h1 = 2 * g, 2 * g + 1
                    op0 = opool.tile([DH + 1, R], F32, tag="op0")
                    op1 = opool.tile([DH + 1, R], F32, tag="op1")
                    sp_cur, pe_cur = emit_qk(g, 0)
                    for sb in range(NSB):
                        sp, on_pe = sp_cur, pe_cur
                        bap = bias_t[:, sb, :]
                        brep = bass.AP(tensor=bap.tensor, offset=bap.offset,
                                       ap=[bap.ap[0], [0, 2]] + bap.ap[1:])
                        if not on_pe:
                            nc.vector.tensor_add(out=sp, in0=sp,
                                                 in1=brep.bitcast(F32))
                        et = ering.tile([SB, 2 * R], F32R, tag="et")
                        nc.scalar.activation(out=et, in_=sp, func=EXPF)
                        if sb + 1 < NSB:
                            sp_cur, pe_cur = emit_qk(g, sb + 1)
                        nc.tensor.matmul(op0, v_t[:, sb, h0, :], et[:, 0:R],
                                         start=(sb == 0), stop=(sb == NSB - 1))
                        nc.tensor.matmul(op1, v_t[:, sb, h1, :], et[:, R:2 * R],
                                         start=(sb == 0), stop=(sb == NSB - 1))
                    for j, op in ((0, op0), (1, op1)):
                        h = 2 * g + j
                        inv = small.tile([1, R], F32R, tag="inv")
                        with nc.allow_low_precision(reason="f32r rounding of softmax recip"):
                            nc.vector.reciprocal(out=inv, in_=op[DH:DH + 1, :])
                        ib = spool.tile([DH, R], F32, tag="sp")
                        nc.tensor.matmul(ib, ones_t[0:1, 0:DH], inv, start=True, stop=True)
                        ibs = small.tile([DH, R], F32R, tag="ibs")
                        nc.scalar.copy(out=ibs, in_=ib)
                        nc.vector.tensor_mul(out=o_t[:, h, :], in0=op[0:DH, :], in1=ibs)

                for rt in range(R // SB):
                    yp = spool.tile([SB, HID], F32, tag="sp")
                    for h in range(NH):
                        nc.tensor.matmul(yp, o_t[:, h, rt * SB:(rt + 1) * SB],
                                         wo_t[:, h, :], start=(h == 0), stop=(h == NH - 1))
                    ys = small.tile([SB, HID], F32, tag="ys")
                    nc.vector.tensor_add(out=ys, in0=yp, in1=bo_t)
                    nc.scalar.dma_start(out=y[rt * SB:(rt + 1) * SB, :], in_=ys)

            if reps == 1:
                attn_body()
            else:
                if not dma_in_loop:
                    bias_dma()
                with tc.For_i(0, reps, 1) as iv:
                    attn_body(iv)

    nc.compile()
    return nc


def prepare_inputs(node_features, temporal_data, edge_index,
                   W_in, b_in, Wq, bq, Wk, bk, Wv, bv, Wo, bo):
    """Host-side prep: layouts, bias matrix, per-core sharding."""
    f32 = np.float32
    nf = np.asarray(node_features, f32)
    W_in = np.asarray(W_in, f32)
    b_in = np.asarray(b_in, f32)
    Wq = np.asarray(Wq, f32)
    bq = np.asarray(bq, f32)
    Wk = np.asarray(Wk, f32)
    bk = np.asarray(bk, f32)
    Wv = np.asarray(Wv, f32)
    bv = np.asarray(bv, f32)
    Wo = np.asarray(Wo, f32)
    bo = np.asarray(bo, f32)
    edge_index = np.asarray(edge_index)

    scale = f32(1.0 / np.sqrt(DH))

    nf_aug_t = np.concatenate([nf.T, np.ones((1, L), f32)], axis=0)   # [46, L]
    w_in_aug = np.concatenate([W_in, b_in[None, :]], axis=0)          # [46, 256]
    wq_s = (Wq * scale).reshape(2, 128, HID)
    wk_r = Wk.reshape(2, 128, HID)
    wv_r = Wv.reshape(2, 128, HID)
    wo_r = Wo.reshape(NH, DH, HID)
    # head-pair packed per-partition biases: [128, 2] (pair g in column g)
    bq_s = np.ascontiguousarray((bq * scale).reshape(2, 128).T)
    bk_r = np.ascontiguousarray(bk.reshape(2, 128).T)
    ones128 = np.ones((1, 128), f32)

    # combined additive bias (time bias where allowed, -1e9 where blocked)
    dt = np.ascontiguousarray(np.asarray(temporal_data[:, :, 0], f32))
    tb = np.zeros_like(dt)
    for lo, hi, w in BUCKETS:
        if w != 0.0:
            tb += f32(w) * ((dt >= f32(lo)) & (dt <= f32(hi))).astype(f32)
    allowed = np.zeros((L, L), dtype=bool)
    allowed[edge_index[0], edge_index[1]] = True
    np.fill_diagonal(allowed, True)
    biasf = np.where(allowed, tb, NEG)            # [r, s]
    biasf_T = np.ascontiguousarray(biasf.T)       # [s, r]

    in_maps = []
    for c in range(NCORES):
        rows = slice(c * R, (c + 1) * R)
        bt = biasf_T[:, rows]                                  # [4096, 512]
        bt = np.ascontiguousarray(
            bt.reshape(NSB, SB, R).transpose(1, 0, 2))         # [128, 32, 512]
        in_maps.append({
            "nf_t": nf_aug_t,
            "nfr_t": np.ascontiguousarray(nf_aug_t[:, rows]),
            "w_in": w_in_aug,
            "wq": wq_s, "wk": wk_r, "wv": wv_r, "wo": wo_r,
            "bqd": bq_s, "bkd": bk_r,
            "bvd": bv[None, :], "bod": bo[None, :],
            "onesd": ones128,
            "identd": np.eye(128, dtype=f32),
            "biasT": bt,
        })
    return in_maps


_PROGRAM = None


def kernel(**inputs) -> np.ndarray:
    global _PROGRAM
    from concourse.bass_utils import run_bass_kernel_spmd

    if _PROGRAM is None:
        _PROGRAM = build_program()
    in_maps = prepare_inputs(**inputs)
    res = run_bass_kernel_spmd(_PROGRAM, in_maps, list(range(NCORES))).results
    return np.concatenate([r["y"] for r in res], axis=0)
